# revision 8
# baseline (speedup 1.0000x reference)
"""Trainium2 Bass kernel for nn_AttentionBlock (Reformer-style LSH attention).

Sharding: 8 cores; core c owns batch c//4 and 4 heads (4*(c%4)..+4).
Device computes all dense math in 3 SPMD dispatches:
  D1: Q/V projections (qT/vT, feature-major)
  D2: per-(head,round) sorted chunk attention: scoresT matmuls, exp,
      multiplicative mask, P@V + row-sum matmuls
  D3: output projection partials (summed on host across cores per batch)
Host does layernorm + permutation bookkeeping (LSH bucket argsort, slab
packing, round combine) — the data-dependent control flow.
"""
import json as _json
import numpy as np

import concourse.bass as bass
import concourse.mybir as mybir
import concourse.tile as tile
from bass_rust import ScopedClock, VectorClock
from concourse.bass_utils import run_bass_kernel_spmd

B, L, D, HEAD, ROUNDS, C = 2, 4096, 1024, 16, 4, 64
DK = D // HEAD          # 64
NB = L // C             # 64 buckets
NCHUNK = L // C         # 64 chunks
HPC = 4                 # heads per core
JOBS = HPC * ROUNDS     # 16 jobs per core

# ---------------------------------------------------------------------------
# runtime patches: this walrus allows only ONE sync wait per instruction.
_MAXW = 1

def _patched_drain(self, tick_clock, wait_clock):
    g = tick_clock.global_clock
    ticks = eval(repr(g).replace("VectorClock(", "").rstrip(")"))
    procs = [(i, t) for i, t in enumerate(ticks) if t > 0]
    for cs in range(0, len(procs), _MAXW):
        sub = VectorClock()
        for i, t in procs[cs:cs + _MAXW]:
            sub.require_at_least(i, t)
        d = self.nc.sync.drain()
        wait_clock.add_sem_waits(d.ins, ScopedClock({None: sub}))
    self.nc.all_engine_barrier()
    popped = self.nc._tile_sem_poison_stack.pop()
    assert popped is self._sem_poison
    self.nc.clear_and_free_semaphores(list(self.sems.allocated().values()))
    self.nc.all_engine_barrier()

tile.TileContext._drain_and_barrier = _patched_drain

_orig_to_json_bytes = bass.Bass.to_json_bytes

def _split_waits(self):
    j = _json.loads(_orig_to_json_bytes(self))
    ctr = 0
    for f in j["functions"]:
        for bb in f["blocks"]:
            new = []
            for ins in bb["instructions"]:
                si = ins.get("sync_info") or {}
                sw = si.get("on_wait") or []
                if len(sw) > 1:
                    for w in sw[:-1]:
                        new.append({"debug": ins.get("debug", 0),
                                    "engine": ins.get("engine"), "ins": [],
                                    "name": f"waitsplit_{ctr}",
                                    "opcode": "EventSemaphore", "outs": [],
                                    "sync_info": {"on_update": [],
                                                  "on_wait": [w]}})
                        ctr += 1
                    si["on_wait"] = [sw[-1]]
                new.append(ins)
            bb["instructions"] = new
    return _json.dumps(j).encode()

bass.Bass.to_json_bytes = _split_waits

F32 = mybir.dt.float32


# ---------------------------------------------------------------------------
def _build_d1():
    """qvT[512, 4096] = [Wq_c | Wv_c]^T @ normT  (per core)."""
    nc = bass.Bass()
    normT = nc.dram_tensor("normT", (D, L), F32, kind="ExternalInput")
    w = nc.dram_tensor("w", (D, 512), F32, kind="ExternalInput")
    qvT = nc.dram_tensor("qvT", (512, L), F32, kind="ExternalOutput")
    with tile.TileContext(nc) as tc:
        with tc.tile_pool(name="wp", bufs=1) as wp, \
             tc.tile_pool(name="xp", bufs=2) as xp, \
             tc.tile_pool(name="op", bufs=2) as op, \
             tc.tile_pool(name="ps", bufs=2, space="PSUM") as ps:
            wt = []
            for k in range(8):
                t = wp.tile([128, 512], F32, tag=f"w{k}")
                nc.sync.dma_start(out=t[:], in_=w[128 * k:128 * (k + 1), :])
                wt.append(t)
            for ch in range(8):
                xt = []
                for k in range(8):
                    t = xp.tile([128, 512], F32, tag=f"x{k}")
                    nc.sync.dma_start(
                        out=t[:], in_=normT[128 * k:128 * (k + 1),
                                            512 * ch:512 * (ch + 1)])
                    xt.append(t)
                for m in range(4):
                    p = ps.tile([128, 512], F32, space="PSUM", tag="p")
                    for k in range(8):
                        nc.tensor.matmul(p[:], lhsT=wt[k][:, 128 * m:128 * (m + 1)],
                                         rhs=xt[k][:], start=(k == 0),
                                         stop=(k == 7))
                    o = op.tile([128, 512], F32, tag="o")
                    nc.scalar.copy(out=o[:], in_=p[:])
                    nc.sync.dma_start(
                        out=qvT[128 * m:128 * (m + 1), 512 * ch:512 * (ch + 1)],
                        in_=o[:])
    return nc


def _build_d2():
    """Sorted chunk attention for 16 jobs.

    kq[j]: [128, 4160]  rows 0:64 = sorted-normalized-K^T (wrap-ext),
                        rows 64:128 = sorted Q^T/8 shifted by 64 cols.
    v[j]:  [128, 33, 64] v_ext rows (row r at [r%128, r//128]).
    m[j]:  [128, 4096]  multiplicative {0,1} mask, chunk n at cols 64n.
    out[j]: [64, 4160]  cols 0:4096 chunk outs, 4096: row sums.
    """
    nc = bass.Bass()
    kin = nc.dram_tensor("kt", (JOBS, 64, 4160), F32, kind="ExternalInput")
    qin = nc.dram_tensor("qt", (JOBS, 64, 4160), F32, kind="ExternalInput")
    va = nc.dram_tensor("va", (JOBS, 128, 33 * 64), F32, kind="ExternalInput")
    vb = nc.dram_tensor("vb", (JOBS, 128, 33 * 64), F32, kind="ExternalInput")
    msk = nc.dram_tensor("m", (JOBS, 128, 4096), F32, kind="ExternalInput")
    ones = nc.dram_tensor("ones", (128, 1), F32, kind="ExternalInput")
    out = nc.dram_tensor("out", (JOBS, 64, 4160), F32, kind="ExternalOutput")
    with tile.TileContext(nc) as tc:
        with tc.tile_pool(name="cst", bufs=1) as cst, \
             tc.tile_pool(name="sb", bufs=2) as sb, \
             tc.tile_pool(name="wk", bufs=3) as wk, \
             tc.tile_pool(name="ps", bufs=2, space="PSUM") as ps:
            onet = cst.tile([128, 1], F32)
            nc.sync.dma_start(out=onet[:], in_=ones[:])
            for j in range(JOBS):
                ktt = sb.tile([64, 4160], F32, tag="kt")
                qtt = sb.tile([64, 4160], F32, tag="qt")
                vat = sb.tile([128, 33 * 64], F32, tag="va")
                vbt = sb.tile([128, 33 * 64], F32, tag="vb")
                mt = sb.tile([128, 4096], F32, tag="m")
                ob = sb.tile([64, 4160], F32, tag="ob")
                nc.sync.dma_start(out=ktt[:], in_=kin[j])
                nc.sync.dma_start(out=qtt[:], in_=qin[j])
                nc.sync.dma_start(out=vat[:], in_=va[j])
                nc.sync.dma_start(out=vbt[:], in_=vb[j])
                nc.sync.dma_start(out=mt[:], in_=msk[j])
                va3 = vat[:].rearrange("p (b d) -> p b d", d=64)
                vb3 = vbt[:].rearrange("p (b d) -> p b d", d=64)
                for g in range(8):
                    ps_s = ps.tile([128, 512], F32, space="PSUM", tag="s")
                    for i in range(8):
                        n = 8 * g + i
                        nc.tensor.matmul(
                            ps_s[:, 64 * i:64 * (i + 1)],
                            lhsT=ktt[:, 64 * n:64 * n + 128],
                            rhs=qtt[:, 64 + 64 * n:128 + 64 * n],
                            start=True, stop=True)
                    es = wk.tile([128, 512], F32, tag="es")
                    nc.scalar.activation(es[:], ps_s[:],
                                         mybir.ActivationFunctionType.Exp)
                    pm = wk.tile([128, 512], F32, tag="pm")
                    nc.vector.scalar_tensor_tensor(
                        out=pm[:], in0=es[:], scalar=1.0,
                        in1=mt[:, 512 * g:512 * (g + 1)],
                        op0=mybir.AluOpType.mult, op1=mybir.AluOpType.mult)
                    ps_o = ps.tile([64, 512], F32, space="PSUM", tag="o")
                    ps_u = ps.tile([64, 8], F32, space="PSUM", tag="u")
                    for i in range(8):
                        n = 8 * g + i
                        dst = ps_o[:, 64 * i:64 * (i + 1)]
                        if n % 2 == 0:
                            nc.tensor.matmul(dst, lhsT=pm[:, 64 * i:64 * (i + 1)],
                                             rhs=va3[:, n // 2, :],
                                             start=True, stop=True)
                        else:
                            nc.tensor.matmul(dst, lhsT=pm[:, 64 * i:64 * (i + 1)],
                                             rhs=vb3[:, (n - 1) // 2, :],
                                             start=True, stop=True)
                        nc.tensor.matmul(ps_u[:, i:i + 1],
                                         lhsT=pm[:, 64 * i:64 * (i + 1)],
                                         rhs=onet[:], start=True, stop=True)
                    nc.scalar.copy(out=ob[:, 512 * g:512 * (g + 1)], in_=ps_o[:])
                    nc.vector.tensor_copy(out=ob[:, 4096 + 8 * g:4096 + 8 * (g + 1)],
                                          in_=ps_u[:])
                nc.sync.dma_start(out=out[j], in_=ob[:])
    return nc


def _build_d3():
    """partial[4096, 1024] = attnT^T @ Wo_c  (per core)."""
    nc = bass.Bass()
    at = nc.dram_tensor("attnT", (256, L), F32, kind="ExternalInput")
    wo = nc.dram_tensor("wo", (256, D), F32, kind="ExternalInput")
    pr = nc.dram_tensor("partial", (L, D), F32, kind="ExternalOutput")
    with tile.TileContext(nc) as tc:
        with tc.tile_pool(name="cp", bufs=1) as cp, \
             tc.tile_pool(name="op", bufs=3) as op, \
             tc.tile_pool(name="ps", bufs=3, space="PSUM") as ps:
            a0 = cp.tile([128, L], F32, tag="a0")
            a1 = cp.tile([128, L], F32, tag="a1")
            w0 = cp.tile([128, D], F32, tag="w0")
            w1 = cp.tile([128, D], F32, tag="w1")
            nc.sync.dma_start(out=a0[:], in_=at[0:128, :])
            nc.sync.dma_start(out=a1[:], in_=at[128:256, :])
            nc.sync.dma_start(out=w0[:], in_=wo[0:128, :])
            nc.sync.dma_start(out=w1[:], in_=wo[128:256, :])
            for mt in range(32):
                for nh in range(2):
                    p = ps.tile([128, 512], F32, space="PSUM", tag="p")
                    nc.tensor.matmul(p[:], lhsT=a0[:, 128 * mt:128 * (mt + 1)],
                                     rhs=w0[:, 512 * nh:512 * (nh + 1)],
                                     start=True, stop=False)
                    nc.tensor.matmul(p[:], lhsT=a1[:, 128 * mt:128 * (mt + 1)],
                                     rhs=w1[:, 512 * nh:512 * (nh + 1)],
                                     start=False, stop=True)
                    o = op.tile([128, 512], F32, tag="o")
                    nc.scalar.copy(out=o[:], in_=p[:])
                    nc.sync.dma_start(
                        out=pr[128 * mt:128 * (mt + 1), 512 * nh:512 * (nh + 1)],
                        in_=o[:])
    return nc


LAST_HW_NS = 0
_DISPATCH_WALLS = []


def _run(nc, in_maps):
    import time as _t
    t0 = _t.time()
    r = run_bass_kernel_spmd(nc, in_maps, core_ids=list(range(8)),
                             trace=False).results
    _DISPATCH_WALLS.append(_t.time() - t0)
    return r


# ---------------------------------------------------------------------------
def kernel(x, Wq, bq, Wv, bv, Wo, bo, gamma, beta, rotations, mask, seed):
    x = np.asarray(x, np.float32)
    Wq = np.asarray(Wq, np.float32); bq = np.asarray(bq, np.float32)
    Wv = np.asarray(Wv, np.float32); bv = np.asarray(bv, np.float32)
    Wo = np.asarray(Wo, np.float32); bo = np.asarray(bo, np.float32)
    gamma = np.asarray(gamma, np.float32); beta = np.asarray(beta, np.float32)
    rotations = np.asarray(rotations, np.float32)
    maskb = np.asarray(mask, bool)

    # host: layernorm (+ affine), feature-major per batch
    mu = x.mean(-1, keepdims=True)
    var = x.var(-1, keepdims=True)
    norm = (x - mu) / np.sqrt(var + 1e-5) * gamma + beta
    normT = np.ascontiguousarray(norm.transpose(0, 2, 1))       # [B, D, L]

    core_b = [c // 4 for c in range(8)]
    core_h0 = [4 * (c % 4) for c in range(8)]

    # ---- D1: projections ----
    d1 = _build_d1()
    in1 = []
    for c in range(8):
        h0 = core_h0[c]
        wc = np.concatenate([Wq[:, 64 * h0:64 * (h0 + 4)],
                             Wv[:, 64 * h0:64 * (h0 + 4)]], axis=1)
        in1.append({"normT": np.ascontiguousarray(normT[core_b[c]]),
                    "w": np.ascontiguousarray(wc)})
    r1 = _run(d1, in1)

    qT = np.zeros((B, HEAD, DK, L), np.float32)
    vT = np.zeros((B, HEAD, DK, L), np.float32)
    for c in range(8):
        qv = r1[c]["qvT"]                                        # [512, L]
        b_, h0 = core_b[c], core_h0[c]
        for hl in range(HPC):
            h = h0 + hl
            qT[b_, h] = qv[64 * hl:64 * (hl + 1)] + bq[64 * h:64 * (h + 1)][:, None]
            vT[b_, h] = qv[256 + 64 * hl:256 + 64 * (hl + 1)] + \
                bv[64 * h:64 * (h + 1)][:, None]

    # host: buckets + stable sort metadata
    rot2 = np.concatenate([rotations, -rotations], axis=2)       # [R, DK, NB]
    pos = np.arange(L)
    slot = np.arange(L)
    tickers = np.zeros((B, HEAD, ROUNDS, L), np.int64)
    kt_all = np.zeros((8, JOBS, 64, 4160), np.float32)
    qt_all = np.zeros((8, JOBS, 64, 4160), np.float32)
    va_all = np.zeros((8, JOBS, 128, 33 * 64), np.float32)
    vb_all = np.zeros((8, JOBS, 128, 33 * 64), np.float32)
    m_all = np.zeros((8, JOBS, 128, 4096), np.float32)
    sv_store = np.zeros((8, JOBS, L, DK), np.float32)

    jq = slot % C                                               # q idx in chunk
    for c in range(8):
        b_, h0 = core_b[c], core_h0[c]
        for hl in range(HPC):
            h = h0 + hl
            q_h = qT[b_, h].T                                    # [L, DK]
            v_h = vT[b_, h].T
            for r in range(ROUNDS):
                j = hl * ROUNDS + r
                scores_rot = q_h @ rot2[r]                       # [L, NB]
                buckets = np.argmax(scores_rot, axis=1)
                tick = np.argsort(buckets * L + pos, kind="stable")
                tickers[b_, h, r] = tick
                sq = q_h[tick]                                   # [L, DK]
                sk = sq / (np.linalg.norm(sq, axis=1, keepdims=True) + 1e-9)
                sv = v_h[tick]
                sv_store[c, j] = sv
                sb_ = buckets[tick]
                # slabs
                kt_all[c, j] = np.concatenate([sk[-C:], sk], axis=0).T
                qt_all[c, j, :, 64:] = sq.T / 8.0
                vext = np.zeros((33 * 128 + C, DK), np.float32)
                vext[0:C] = sv[-C:]
                vext[C:C + L] = sv
                va_all[c, j] = vext[:33 * 128].reshape(33, 128, DK) \
                    .transpose(1, 0, 2).reshape(128, 33 * 64)
                vextb = np.zeros((33 * 128, DK), np.float32)
                vextb[:33 * 128 - C] = vext[C:33 * 128]
                vb_all[c, j] = vextb.reshape(33, 128, DK) \
                    .transpose(1, 0, 2).reshape(128, 33 * 64)
                # mask: chunk n key t=64(n-1)+jj (global slot, wrap kills),
                # query s=64n+qi ; valid = same bucket & t<=s & t!=s & km
                sb_ext = np.concatenate([sb_[-C:], sb_])
                km_ext = np.concatenate([maskb[b_][tick][-C:], maskb[b_][tick]])
                n_idx = np.arange(NCHUNK)[:, None, None]
                jj = np.arange(128)[None, :, None]
                qi = np.arange(64)[None, None, :]
                tglob = 64 * (n_idx - 1) + jj                     # <0 => wrap
                sglob = 64 * n_idx + qi
                ext_idx = 64 * n_idx + jj                         # index in *_ext
                samebucket = sb_ext[ext_idx] == sb_[sglob]
                kmv = km_ext[ext_idx]
                valid = samebucket & (tglob >= 0) & (tglob <= sglob) & \
                    (tglob != sglob) & kmv
                # wrap region (n=0, jj<64): tglob<0 -> actual slot large -> inv
                mfull = valid.astype(np.float32)                  # [64, 128, 64]
                m_all[c, j] = mfull.transpose(1, 0, 2).reshape(128, 4096)

    # ---- D2: attention ----
    d2 = _build_d2()
    ones = np.ones((128, 1), np.float32)
    in2 = [{"kt": kt_all[c], "qt": qt_all[c], "va": va_all[c],
            "vb": vb_all[c], "m": m_all[c], "ones": ones}
           for c in range(8)]
    r2 = _run(d2, in2)

    # host: normalize, fallback, unsort, combine rounds
    attnT = np.zeros((8, 256, L), np.float32)
    for c in range(8):
        b_, h0 = core_b[c], core_h0[c]
        raw = r2[c]["out"]                                       # [16, 64, 4160]
        for hl in range(HPC):
            h = h0 + hl
            outs_tok = np.zeros((ROUNDS, L, DK), np.float32)
            lse_tok = np.zeros((ROUNDS, L), np.float32)
            for r in range(ROUNDS):
                j = hl * ROUNDS + r
                rb = raw[j]                                      # [64, 4160]
                o_sorted = np.zeros((L, DK), np.float32)
                sums = np.zeros(L, np.float32)
                for g in range(8):
                    blk = rb[:, 512 * g:512 * (g + 1)].reshape(64, 8, 64)
                    for i in range(8):
                        n = 8 * g + i
                        o_sorted[64 * n:64 * (n + 1)] = blk[:, i, :]
                    sums[64 * 8 * g:64 * 8 * (g + 1)] = \
                        rb[:, 4096 + 8 * g:4096 + 8 * (g + 1)].T.reshape(-1)
                only_self = sums <= 0.0
                safe = np.where(only_self, 1.0, sums)
                o_n = o_sorted / safe[:, None]
                o_n[only_self] = sv_store[c, hl * ROUNDS + r][only_self]
                lse_s = np.where(only_self, -1e5, np.log(safe))
                tick = tickers[b_, h, r]
                o_tok = np.zeros_like(o_n); o_tok[tick] = o_n
                l_tok = np.zeros_like(lse_s); l_tok[tick] = lse_s
                outs_tok[r] = o_tok
                lse_tok[r] = l_tok
            w = lse_tok - lse_tok.max(0, keepdims=True)
            w = np.exp(w); w /= w.sum(0, keepdims=True)
            attnT[c, 64 * hl:64 * (hl + 1)] = \
                np.einsum("rl,rld->dl", w, outs_tok)

    # ---- D3: output projection ----
    d3 = _build_d3()
    in3 = []
    for c in range(8):
        h0 = core_h0[c]
        in3.append({"attnT": np.ascontiguousarray(attnT[c]),
                    "wo": np.ascontiguousarray(Wo[64 * h0:64 * (h0 + 4), :])})
    r3 = _run(d3, in3)

    out = np.zeros((B, L, D), np.float32)
    for c in range(8):
        out[core_b[c]] += r3[c]["partial"]
    out += bo
    return out


# revision 10
# speedup vs baseline: 1.1338x; 1.1338x over previous
"""Trainium2 Bass kernel for nn_AttentionBlock (Reformer-style LSH attention).

Sharding: 8 cores; core c owns batch c//4 and 4 heads (4*(c%4)..+4).
Device computes all dense math in 3 SPMD dispatches:
  D1: Q/V projections (qT/vT, feature-major)
  D2: per-(head,round) sorted chunk attention: scoresT matmuls, exp,
      multiplicative mask, P@V + row-sum matmuls
  D3: output projection partials (summed on host across cores per batch)
Host does layernorm + permutation bookkeeping (LSH bucket argsort, slab
packing, round combine) — the data-dependent control flow.
"""
import json as _json
import numpy as np

import concourse.bass as bass
import concourse.mybir as mybir
import concourse.tile as tile
from bass_rust import ScopedClock, VectorClock
from concourse.bass_utils import run_bass_kernel_spmd

B, L, D, HEAD, ROUNDS, C = 2, 4096, 1024, 16, 4, 64
DK = D // HEAD          # 64
NB = L // C             # 64 buckets
NCHUNK = L // C         # 64 chunks
HPC = 4                 # heads per core
JOBS = HPC * ROUNDS     # 16 jobs per core

# ---------------------------------------------------------------------------
# runtime patches: this walrus allows only ONE sync wait per instruction.
_MAXW = 1

def _patched_drain(self, tick_clock, wait_clock):
    g = tick_clock.global_clock
    ticks = eval(repr(g).replace("VectorClock(", "").rstrip(")"))
    procs = [(i, t) for i, t in enumerate(ticks) if t > 0]
    for cs in range(0, len(procs), _MAXW):
        sub = VectorClock()
        for i, t in procs[cs:cs + _MAXW]:
            sub.require_at_least(i, t)
        d = self.nc.sync.drain()
        wait_clock.add_sem_waits(d.ins, ScopedClock({None: sub}))
    self.nc.all_engine_barrier()
    popped = self.nc._tile_sem_poison_stack.pop()
    assert popped is self._sem_poison
    self.nc.clear_and_free_semaphores(list(self.sems.allocated().values()))
    self.nc.all_engine_barrier()

tile.TileContext._drain_and_barrier = _patched_drain

_orig_to_json_bytes = bass.Bass.to_json_bytes

def _split_waits(self):
    j = _json.loads(_orig_to_json_bytes(self))
    ctr = 0
    for f in j["functions"]:
        for bb in f["blocks"]:
            new = []
            for ins in bb["instructions"]:
                si = ins.get("sync_info") or {}
                sw = si.get("on_wait") or []
                if len(sw) > 1:
                    for w in sw[:-1]:
                        new.append({"debug": ins.get("debug", 0),
                                    "engine": ins.get("engine"), "ins": [],
                                    "name": f"waitsplit_{ctr}",
                                    "opcode": "EventSemaphore", "outs": [],
                                    "sync_info": {"on_update": [],
                                                  "on_wait": [w]}})
                        ctr += 1
                    si["on_wait"] = [sw[-1]]
                new.append(ins)
            bb["instructions"] = new
    return _json.dumps(j).encode()

bass.Bass.to_json_bytes = _split_waits

F32 = mybir.dt.float32


# ---------------------------------------------------------------------------
def _build_d1():
    """qvT[512, 4096] = [Wq_c | Wv_c]^T @ normT  (per core)."""
    nc = bass.Bass()
    normT = nc.dram_tensor("normT", (D, L), F32, kind="ExternalInput")
    w = nc.dram_tensor("w", (D, 512), F32, kind="ExternalInput")
    qvT = nc.dram_tensor("qvT", (512, L), F32, kind="ExternalOutput")
    with tile.TileContext(nc) as tc:
        with tc.tile_pool(name="wp", bufs=1) as wp, \
             tc.tile_pool(name="xp", bufs=2) as xp, \
             tc.tile_pool(name="op", bufs=2) as op, \
             tc.tile_pool(name="ps", bufs=2, space="PSUM") as ps:
            wt = []
            for k in range(8):
                t = wp.tile([128, 512], F32, tag=f"w{k}")
                nc.sync.dma_start(out=t[:], in_=w[128 * k:128 * (k + 1), :])
                wt.append(t)
            for ch in range(8):
                xt = []
                for k in range(8):
                    t = xp.tile([128, 512], F32, tag=f"x{k}")
                    nc.sync.dma_start(
                        out=t[:], in_=normT[128 * k:128 * (k + 1),
                                            512 * ch:512 * (ch + 1)])
                    xt.append(t)
                for m in range(4):
                    p = ps.tile([128, 512], F32, space="PSUM", tag="p")
                    for k in range(8):
                        nc.tensor.matmul(p[:], lhsT=wt[k][:, 128 * m:128 * (m + 1)],
                                         rhs=xt[k][:], start=(k == 0),
                                         stop=(k == 7))
                    o = op.tile([128, 512], F32, tag="o")
                    nc.scalar.copy(out=o[:], in_=p[:])
                    nc.sync.dma_start(
                        out=qvT[128 * m:128 * (m + 1), 512 * ch:512 * (ch + 1)],
                        in_=o[:])
    return nc


def _build_d2():
    """Sorted chunk attention for 16 jobs.

    kq[j]: [128, 4160]  rows 0:64 = sorted-normalized-K^T (wrap-ext),
                        rows 64:128 = sorted Q^T/8 shifted by 64 cols.
    v[j]:  [128, 33, 64] v_ext rows (row r at [r%128, r//128]).
    m[j]:  [128, 4096]  multiplicative {0,1} mask, chunk n at cols 64n.
    out[j]: [64, 4160]  cols 0:4096 chunk outs, 4096: row sums.
    """
    nc = bass.Bass()
    kin = nc.dram_tensor("kt", (JOBS, 64, 4160), F32, kind="ExternalInput")
    qin = nc.dram_tensor("qt", (JOBS, 64, 4160), F32, kind="ExternalInput")
    va = nc.dram_tensor("va", (JOBS, 128, 33 * 64), F32, kind="ExternalInput")
    vb = nc.dram_tensor("vb", (JOBS, 128, 33 * 64), F32, kind="ExternalInput")
    msk = nc.dram_tensor("m", (JOBS, 128, 4096), F32, kind="ExternalInput")
    ones = nc.dram_tensor("ones", (128, 1), F32, kind="ExternalInput")
    out = nc.dram_tensor("out", (JOBS, 64, 4160), F32, kind="ExternalOutput")
    with tile.TileContext(nc) as tc:
        with tc.tile_pool(name="cst", bufs=1) as cst, \
             tc.tile_pool(name="sb", bufs=2) as sb, \
             tc.tile_pool(name="wk", bufs=4) as wk, \
             tc.tile_pool(name="ps", bufs=2, space="PSUM") as ps, \
             tc.tile_pool(name="ps3", bufs=3, space="PSUM") as ps3:
            onet = cst.tile([128, 1], F32)
            nc.sync.dma_start(out=onet[:], in_=ones[:])
            for j in range(JOBS):
                ktt = sb.tile([64, 4160], F32, tag="kt")
                qtt = sb.tile([64, 4160], F32, tag="qt")
                vat = sb.tile([128, 33 * 64], F32, tag="va")
                vbt = sb.tile([128, 33 * 64], F32, tag="vb")
                mt = sb.tile([128, 4096], F32, tag="m")
                ob = sb.tile([64, 4160], F32, tag="ob")
                nc.sync.dma_start(out=ktt[:], in_=kin[j])
                nc.sync.dma_start(out=qtt[:], in_=qin[j])
                nc.sync.dma_start(out=vat[:], in_=va[j])
                nc.sync.dma_start(out=vbt[:], in_=vb[j])
                nc.sync.dma_start(out=mt[:], in_=msk[j])
                va3 = vat[:].rearrange("p (b d) -> p b d", d=64)
                vb3 = vbt[:].rearrange("p (b d) -> p b d", d=64)
                for g in range(8):
                    ps_s = ps3.tile([128, 512], F32, space="PSUM", tag="s")
                    for i in range(8):
                        n = 8 * g + i
                        nc.tensor.matmul(
                            ps_s[:, 64 * i:64 * (i + 1)],
                            lhsT=ktt[:, 64 * n:64 * n + 128],
                            rhs=qtt[:, 64 + 64 * n:128 + 64 * n],
                            start=True, stop=True)
                    es = wk.tile([128, 512], F32, tag="es")
                    nc.scalar.activation(es[:], ps_s[:],
                                         mybir.ActivationFunctionType.Exp)
                    pm = wk.tile([128, 512], F32, tag="pm")
                    nc.vector.scalar_tensor_tensor(
                        out=pm[:], in0=es[:], scalar=1.0,
                        in1=mt[:, 512 * g:512 * (g + 1)],
                        op0=mybir.AluOpType.mult, op1=mybir.AluOpType.mult)
                    ps_o = ps.tile([64, 512], F32, space="PSUM", tag="o")
                    ps_u = ps.tile([64, 8], F32, space="PSUM", tag="u")
                    for i in range(8):
                        n = 8 * g + i
                        dst = ps_o[:, 64 * i:64 * (i + 1)]
                        if n % 2 == 0:
                            nc.tensor.matmul(dst, lhsT=pm[:, 64 * i:64 * (i + 1)],
                                             rhs=va3[:, n // 2, :],
                                             start=True, stop=True)
                        else:
                            nc.tensor.matmul(dst, lhsT=pm[:, 64 * i:64 * (i + 1)],
                                             rhs=vb3[:, (n - 1) // 2, :],
                                             start=True, stop=True)
                        nc.tensor.matmul(ps_u[:, i:i + 1],
                                         lhsT=pm[:, 64 * i:64 * (i + 1)],
                                         rhs=onet[:], start=True, stop=True)
                    nc.scalar.copy(out=ob[:, 512 * g:512 * (g + 1)], in_=ps_o[:])
                    nc.vector.tensor_copy(out=ob[:, 4096 + 8 * g:4096 + 8 * (g + 1)],
                                          in_=ps_u[:])
                nc.sync.dma_start(out=out[j], in_=ob[:])
    return nc


def _build_d3():
    """partial[4096, 1024] = attnT^T @ Wo_c  (per core)."""
    nc = bass.Bass()
    at = nc.dram_tensor("attnT", (256, L), F32, kind="ExternalInput")
    wo = nc.dram_tensor("wo", (256, D), F32, kind="ExternalInput")
    pr = nc.dram_tensor("partial", (L, D), F32, kind="ExternalOutput")
    with tile.TileContext(nc) as tc:
        with tc.tile_pool(name="cp", bufs=1) as cp, \
             tc.tile_pool(name="op", bufs=3) as op, \
             tc.tile_pool(name="ps", bufs=3, space="PSUM") as ps:
            a0 = cp.tile([128, L], F32, tag="a0")
            a1 = cp.tile([128, L], F32, tag="a1")
            w0 = cp.tile([128, D], F32, tag="w0")
            w1 = cp.tile([128, D], F32, tag="w1")
            nc.sync.dma_start(out=a0[:], in_=at[0:128, :])
            nc.sync.dma_start(out=a1[:], in_=at[128:256, :])
            nc.sync.dma_start(out=w0[:], in_=wo[0:128, :])
            nc.sync.dma_start(out=w1[:], in_=wo[128:256, :])
            for mt in range(32):
                for nh in range(2):
                    p = ps.tile([128, 512], F32, space="PSUM", tag="p")
                    nc.tensor.matmul(p[:], lhsT=a0[:, 128 * mt:128 * (mt + 1)],
                                     rhs=w0[:, 512 * nh:512 * (nh + 1)],
                                     start=True, stop=False)
                    nc.tensor.matmul(p[:], lhsT=a1[:, 128 * mt:128 * (mt + 1)],
                                     rhs=w1[:, 512 * nh:512 * (nh + 1)],
                                     start=False, stop=True)
                    o = op.tile([128, 512], F32, tag="o")
                    nc.scalar.copy(out=o[:], in_=p[:])
                    nc.sync.dma_start(
                        out=pr[128 * mt:128 * (mt + 1), 512 * nh:512 * (nh + 1)],
                        in_=o[:])
    return nc


LAST_HW_NS = 0
_DISPATCH_WALLS = []


def _run(nc, in_maps):
    import time as _t
    t0 = _t.time()
    r = run_bass_kernel_spmd(nc, in_maps, core_ids=list(range(8)),
                             trace=False).results
    _DISPATCH_WALLS.append(_t.time() - t0)
    return r


# ---------------------------------------------------------------------------
def kernel(x, Wq, bq, Wv, bv, Wo, bo, gamma, beta, rotations, mask, seed):
    x = np.asarray(x, np.float32)
    Wq = np.asarray(Wq, np.float32); bq = np.asarray(bq, np.float32)
    Wv = np.asarray(Wv, np.float32); bv = np.asarray(bv, np.float32)
    Wo = np.asarray(Wo, np.float32); bo = np.asarray(bo, np.float32)
    gamma = np.asarray(gamma, np.float32); beta = np.asarray(beta, np.float32)
    rotations = np.asarray(rotations, np.float32)
    maskb = np.asarray(mask, bool)

    # host: layernorm (+ affine), feature-major per batch
    mu = x.mean(-1, keepdims=True)
    var = x.var(-1, keepdims=True)
    norm = (x - mu) / np.sqrt(var + 1e-5) * gamma + beta
    normT = np.ascontiguousarray(norm.transpose(0, 2, 1))       # [B, D, L]

    core_b = [c // 4 for c in range(8)]
    core_h0 = [4 * (c % 4) for c in range(8)]

    # ---- D1: projections ----
    d1 = _build_d1()
    in1 = []
    for c in range(8):
        h0 = core_h0[c]
        wc = np.concatenate([Wq[:, 64 * h0:64 * (h0 + 4)],
                             Wv[:, 64 * h0:64 * (h0 + 4)]], axis=1)
        in1.append({"normT": np.ascontiguousarray(normT[core_b[c]]),
                    "w": np.ascontiguousarray(wc)})
    r1 = _run(d1, in1)

    qT = np.zeros((B, HEAD, DK, L), np.float32)
    vT = np.zeros((B, HEAD, DK, L), np.float32)
    for c in range(8):
        qv = r1[c]["qvT"]                                        # [512, L]
        b_, h0 = core_b[c], core_h0[c]
        for hl in range(HPC):
            h = h0 + hl
            qT[b_, h] = qv[64 * hl:64 * (hl + 1)] + bq[64 * h:64 * (h + 1)][:, None]
            vT[b_, h] = qv[256 + 64 * hl:256 + 64 * (hl + 1)] + \
                bv[64 * h:64 * (h + 1)][:, None]

    # host: buckets + stable sort metadata
    rot2 = np.concatenate([rotations, -rotations], axis=2)       # [R, DK, NB]
    pos = np.arange(L)
    slot = np.arange(L)
    tickers = np.zeros((B, HEAD, ROUNDS, L), np.int64)
    kt_all = np.zeros((8, JOBS, 64, 4160), np.float32)
    qt_all = np.zeros((8, JOBS, 64, 4160), np.float32)
    va_all = np.zeros((8, JOBS, 128, 33 * 64), np.float32)
    vb_all = np.zeros((8, JOBS, 128, 33 * 64), np.float32)
    m_all = np.zeros((8, JOBS, 128, 4096), np.float32)
    sv_store = np.zeros((8, JOBS, L, DK), np.float32)

    jq = slot % C                                               # q idx in chunk
    for c in range(8):
        b_, h0 = core_b[c], core_h0[c]
        for hl in range(HPC):
            h = h0 + hl
            q_h = qT[b_, h].T                                    # [L, DK]
            v_h = vT[b_, h].T
            for r in range(ROUNDS):
                j = hl * ROUNDS + r
                scores_rot = q_h @ rot2[r]                       # [L, NB]
                buckets = np.argmax(scores_rot, axis=1)
                tick = np.argsort(buckets * L + pos, kind="stable")
                tickers[b_, h, r] = tick
                sq = q_h[tick]                                   # [L, DK]
                sk = sq / (np.linalg.norm(sq, axis=1, keepdims=True) + 1e-9)
                sv = v_h[tick]
                sv_store[c, j] = sv
                sb_ = buckets[tick]
                # slabs
                kt_all[c, j] = np.concatenate([sk[-C:], sk], axis=0).T
                qt_all[c, j, :, 64:] = sq.T / 8.0
                vext = np.zeros((33 * 128 + C, DK), np.float32)
                vext[0:C] = sv[-C:]
                vext[C:C + L] = sv
                va_all[c, j] = vext[:33 * 128].reshape(33, 128, DK) \
                    .transpose(1, 0, 2).reshape(128, 33 * 64)
                vextb = np.zeros((33 * 128, DK), np.float32)
                vextb[:33 * 128 - C] = vext[C:33 * 128]
                vb_all[c, j] = vextb.reshape(33, 128, DK) \
                    .transpose(1, 0, 2).reshape(128, 33 * 64)
                # mask: chunk n key t=64(n-1)+jj (global slot, wrap kills),
                # query s=64n+qi ; valid = same bucket & t<=s & t!=s & km
                sb_ext = np.concatenate([sb_[-C:], sb_])
                km_ext = np.concatenate([maskb[b_][tick][-C:], maskb[b_][tick]])
                n_idx = np.arange(NCHUNK)[:, None, None]
                jj = np.arange(128)[None, :, None]
                qi = np.arange(64)[None, None, :]
                tglob = 64 * (n_idx - 1) + jj                     # <0 => wrap
                sglob = 64 * n_idx + qi
                ext_idx = 64 * n_idx + jj                         # index in *_ext
                samebucket = sb_ext[ext_idx] == sb_[sglob]
                kmv = km_ext[ext_idx]
                valid = samebucket & (tglob >= 0) & (tglob <= sglob) & \
                    (tglob != sglob) & kmv
                # wrap region (n=0, jj<64): tglob<0 -> actual slot large -> inv
                mfull = valid.astype(np.float32)                  # [64, 128, 64]
                m_all[c, j] = mfull.transpose(1, 0, 2).reshape(128, 4096)

    # ---- D2: attention ----
    d2 = _build_d2()
    ones = np.ones((128, 1), np.float32)
    in2 = [{"kt": kt_all[c], "qt": qt_all[c], "va": va_all[c],
            "vb": vb_all[c], "m": m_all[c], "ones": ones}
           for c in range(8)]
    r2 = _run(d2, in2)

    # host: normalize, fallback, unsort, combine rounds
    attnT = np.zeros((8, 256, L), np.float32)
    for c in range(8):
        b_, h0 = core_b[c], core_h0[c]
        raw = r2[c]["out"]                                       # [16, 64, 4160]
        for hl in range(HPC):
            h = h0 + hl
            outs_tok = np.zeros((ROUNDS, L, DK), np.float32)
            lse_tok = np.zeros((ROUNDS, L), np.float32)
            for r in range(ROUNDS):
                j = hl * ROUNDS + r
                rb = raw[j]                                      # [64, 4160]
                o_sorted = np.zeros((L, DK), np.float32)
                sums = np.zeros(L, np.float32)
                for g in range(8):
                    blk = rb[:, 512 * g:512 * (g + 1)].reshape(64, 8, 64)
                    for i in range(8):
                        n = 8 * g + i
                        o_sorted[64 * n:64 * (n + 1)] = blk[:, i, :]
                    sums[64 * 8 * g:64 * 8 * (g + 1)] = \
                        rb[:, 4096 + 8 * g:4096 + 8 * (g + 1)].T.reshape(-1)
                only_self = sums <= 0.0
                safe = np.where(only_self, 1.0, sums)
                o_n = o_sorted / safe[:, None]
                o_n[only_self] = sv_store[c, hl * ROUNDS + r][only_self]
                lse_s = np.where(only_self, -1e5, np.log(safe))
                tick = tickers[b_, h, r]
                o_tok = np.zeros_like(o_n); o_tok[tick] = o_n
                l_tok = np.zeros_like(lse_s); l_tok[tick] = lse_s
                outs_tok[r] = o_tok
                lse_tok[r] = l_tok
            w = lse_tok - lse_tok.max(0, keepdims=True)
            w = np.exp(w); w /= w.sum(0, keepdims=True)
            attnT[c, 64 * hl:64 * (hl + 1)] = \
                np.einsum("rl,rld->dl", w, outs_tok)

    # ---- D3: output projection ----
    d3 = _build_d3()
    in3 = []
    for c in range(8):
        h0 = core_h0[c]
        in3.append({"attnT": np.ascontiguousarray(attnT[c]),
                    "wo": np.ascontiguousarray(Wo[64 * h0:64 * (h0 + 4), :])})
    r3 = _run(d3, in3)

    out = np.zeros((B, L, D), np.float32)
    for c in range(8):
        out[core_b[c]] += r3[c]["partial"]
    out += bo
    return out
